# revision 36
# baseline (speedup 1.0000x reference)
"""Multi-head attention (B=2, S=2048, H=1024, 16 heads) on 8 TRN2 NeuronCores.

Sharding: data parallel on batch (2) x tensor parallel on heads (4 heads/core,
Megatron column-split qkv, row-split wo). Host pre-transposes x/y, pre-scales
wq by dh^-0.5, and sum-reduces the 4 partial outputs per batch element.

Per-core kernel:
  Projections (bf16): QT/KT in [head-pair-dims(128) x S] transposed layout,
  V in natural [S x dims] layout with a fused ones column scaled by exp(bias)
  (exact additive-bias support). The K/Q/V projection work is emitted as
  small PSUM groups: a minimal prefix runs before attention starts, the rest
  is woven into the attention k-loop to fill TensorE idle slots while the
  Scalar engine (exp) paces the pipeline.

  Attention per 512-wide q-block and head pair: row-tiled (2-head packed)
  QK^T -> logitsT psum [128,1024] -> one ACT exp per pair (psum->sbuf bf16)
  -> PV matmul with fused denominator row (fp32 accumulate) -> fast
  reciprocal + gpsimd partition_broadcast normalize -> pair-stacked bf16
  output projection, deferred one block for overlap.
"""
import sys
sys.path.insert(0, '/opt/trn_rl_repo')
from collections import deque
from contextlib import ExitStack

import numpy as np
import ml_dtypes

import concourse.bacc as bacc
import concourse.tile as tile
from concourse import mybir
from concourse import bass_utils

B, S, H, NH = 2, 2048, 1024, 16
DH = H // NH            # 64
NCORES = 8
HPC = NH // (NCORES // B)   # 4 heads per core
C = HPC * DH            # 256 projected cols per core
KT_H = H // 128         # 8 contraction tiles over H
SK = S // 128           # 16 s-subtiles
JBLK = 512
NJ = S // JBLK          # 4 q-blocks
F32 = mybir.dt.float32
F32R = mybir.dt.float32r
BF16 = mybir.dt.bfloat16

_CACHE = {}
_DEBUG = False


def _build():
    nc = bacc.Bacc('TRN2', debug=False, num_devices=NCORES)
    xT = nc.dram_tensor('xT', [H, S], BF16, kind='ExternalInput')
    yT = nc.dram_tensor('yT', [H, S], BF16, kind='ExternalInput')
    wq = nc.dram_tensor('wq', [H, C], BF16, kind='ExternalInput')
    wk = nc.dram_tensor('wk', [H, C], BF16, kind='ExternalInput')
    wv = nc.dram_tensor('wv', [H, C], BF16, kind='ExternalInput')
    wo = nc.dram_tensor('wo', [C, H], BF16, kind='ExternalInput')
    ebias = nc.dram_tensor('ebias', [128, SK], F32, kind='ExternalInput')
    out = nc.dram_tensor('out', [S, H], F32, kind='ExternalOutput')

    with tile.TileContext(nc) as tc, ExitStack() as ctx:
        res = ctx.enter_context(tc.tile_pool(name='res', bufs=1))
        expool = ctx.enter_context(tc.tile_pool(name='expool', bufs=4))
        ctxpool = ctx.enter_context(tc.tile_pool(name='ctxpool', bufs=2))
        small = ctx.enter_context(tc.tile_pool(name='small', bufs=3))
        outpool = ctx.enter_context(tc.tile_pool(name='outpool', bufs=3))
        ps_qk = ctx.enter_context(tc.tile_pool(name='ps_qk', bufs=2, space='PSUM'))
        ps_pv = ctx.enter_context(tc.tile_pool(name='ps_pv', bufs=4, space='PSUM'))

        # ---- input DMAs, ordered for earliest compute start ----
        wq_r = res.tile([128, KT_H, C], BF16, tag='wq')
        wk_r = res.tile([128, KT_H, C], BF16, tag='wk')
        wv_r = res.tile([128, KT_H, C], BF16, tag='wv')
        nc.sync.dma_start(out=wk_r, in_=wk.ap().rearrange('(t p) c -> p t c', p=128))
        nc.sync.dma_start(out=wv_r, in_=wv.ap().rearrange('(t p) c -> p t c', p=128))
        nc.sync.dma_start(out=wq_r, in_=wq.ap().rearrange('(t p) c -> p t c', p=128))
        eb = res.tile([128, SK], F32, tag='eb')
        nc.sync.dma_start(out=eb, in_=ebias.ap())
        ones4 = res.tile([128, HPC, 1], F32, tag='ones4')
        nc.vector.memset(ones4, 1.0)

        xT_ap, yT_ap = xT.ap(), yT.ap()
        HB = S // 2
        xts = [[res.tile([128, HB], BF16, tag=f'xts{k}_{j}', name=f'xts{k}_{j}')
                for j in range(2)] for k in range(KT_H)]
        yts = [[res.tile([128, HB], BF16, tag=f'yts{k}_{j}', name=f'yts{k}_{j}')
                for j in range(2)] for k in range(KT_H)]
        for j in range(2):
            hs = slice(j * HB, (j + 1) * HB)
            for k in range(KT_H):
                nc.scalar.dma_start(out=yts[k][j],
                                    in_=yT_ap[k * 128:(k + 1) * 128, hs])
            for k in range(KT_H):
                nc.scalar.dma_start(out=xts[k][j],
                                    in_=xT_ap[k * 128:(k + 1) * 128, hs])
        wo_r = res.tile([128, 2, H], BF16, tag='wo')
        nc.sync.dma_start(out=wo_r, in_=wo.ap().rearrange('(t p) n -> p t n', p=128))

        # ---- resident activations ----
        QT = [res.tile([128, S], BF16, tag=f'qt{p}', name=f'qt{p}') for p in range(2)]
        KTs = [res.tile([128, S], BF16, tag=f'kt{p}', name=f'kt{p}') for p in range(2)]
        v_sb = [res.tile([128, HPC, DH + 1], BF16, tag=f'v{i}', name=f'v{i}')
                for i in range(SK)]

        # ---- projection groups (8 matmuls + eviction), run direct or woven ----
        gid = [0]

        def qk_group(which, p, j4):
            w_r = wq_r if which == 'q' else wk_r
            src = xts if which == 'q' else yts
            dest = QT[p] if which == 'q' else KTs[p]
            js = slice(j4 * JBLK, (j4 + 1) * JBLK)
            hj = slice((j4 % 2) * JBLK, (j4 % 2 + 1) * JBLK)
            cs = slice(p * 128, (p + 1) * 128)
            gid[0] += 1
            ps = ps_pv.tile([128, JBLK], F32, tag='pv', name=f'g{gid[0]}')
            items = []
            for k in range(KT_H):
                def mm(k=k):
                    nc.tensor.matmul(ps, w_r[:, k, cs], src[k][j4 // 2][:, hj],
                                     start=(k == 0), stop=(k == KT_H - 1))
                items.append(mm)

            def fin():
                nc.vector.tensor_copy(dest[:, js], ps)
            items.append(fin)
            return items

        def v_group(j4, m):
            sub = j4 * 4 + m
            hj0 = (j4 % 2) * JBLK + m * 128
            gid[0] += 1
            ps = ps_pv.tile([128, JBLK], F32, tag='pv', name=f'g{gid[0]}')
            items = []
            for k in range(KT_H):
                def mm(k=k):
                    nc.tensor.matmul(ps[:, 0:C],
                                     yts[k][j4 // 2][:, hj0:hj0 + 128],
                                     wv_r[:, k, :],
                                     start=(k == 0), stop=(k == KT_H - 1))
                items.append(mm)

            def fin():
                nc.vector.tensor_scalar_mul(
                    v_sb[sub][:, :, 0:DH],
                    ps[:, 0:C].rearrange('p (h c) -> p h c', h=HPC),
                    eb[:, sub:sub + 1])
                nc.gpsimd.tensor_scalar_mul(v_sb[sub][:, :, DH:DH + 1], ones4,
                                            eb[:, sub:sub + 1])
            items.append(fin)
            return items

        # prefix: everything attention block (J0,p0) touches
        for grp in ([qk_group('k', 0, j4) for j4 in range(NJ)]
                    + [qk_group('q', 0, 0)]
                    + [v_group(j4, m) for j4 in range(NJ) for m in range(4)]):
            for it in grp:
                it()

        # woven into the attention k-loop (2 items/step meets all deadlines)
        weave = deque()
        for grp in ([qk_group('k', 1, 0)]
                    + [qk_group('q', 1, 0)]
                    + [qk_group('k', 1, j4) for j4 in range(1, NJ)]
                    + [qk_group('q', 0, 1), qk_group('q', 1, 1),
                       qk_group('q', 0, 2), qk_group('q', 1, 2),
                       qk_group('q', 0, 3), qk_group('q', 1, 3)]):
            weave.extend(grp)

        def weave_emit(n):
            for _ in range(n):
                if weave:
                    weave.popleft()()

        # ---- attention + output projection ----
        dbg = {}
        if _DEBUG:
            for nm, shp in [('d_ex', [128, 2 * JBLK]), ('d_raw', [128, JBLK]),
                            ('d_ctx', [128, JBLK])]:
                dbg[nm] = nc.dram_tensor(nm, shp, F32, kind='ExternalOutput')

        pend_state = {'g': []}

        def out_groups(J, ctx_tiles):
            groups = []
            for m in range(4):
                for n in range(2):
                    def grp(m=m, n=n):
                        ms = slice(m * 128, (m + 1) * 128)
                        ns = slice(n * JBLK, (n + 1) * JBLK)
                        pso = ps_pv.tile([128, JBLK], F32, tag='pv', name=f'o{J}_{m}_{n}')
                        for p in range(2):
                            nc.tensor.matmul(pso, ctx_tiles[p][:, ms], wo_r[:, p, ns],
                                             start=(p == 0), stop=(p == 1))
                        ob = outpool.tile([128, JBLK], F32, tag='ob')
                        nc.vector.tensor_copy(ob, pso)
                        nc.sync.dma_start(out=out.ap()[J * JBLK + m * 128:
                                                       J * JBLK + (m + 1) * 128, ns],
                                          in_=ob)
                    groups.append(grp)
            return groups

        pairs = [(J, p) for J in range(NJ) for p in range(2)]
        psl_q = deque()

        def emit_qk(pidx, kk):
            if pidx >= len(pairs):
                return
            J, p = pairs[pidx]
            js = slice(J * JBLK, (J + 1) * JBLK)
            kks = slice(kk * 128, (kk + 1) * 128)
            psl = ps_qk.tile([128, 2 * JBLK], F32, tag='qk',
                             name=f'psl{pidx}_{kk}')
            nc.tensor.matmul(psl[:, 0:JBLK],
                             KTs[p][0:64, kks], QT[p][0:64, js],
                             start=True, stop=True, tile_position=(0, 0))
            nc.tensor.matmul(psl[:, JBLK:2 * JBLK],
                             KTs[p][64:128, kks], QT[p][64:128, js],
                             start=True, stop=True, tile_position=(64, 0))
            psl_q.append(psl)

        emit_qk(0, 0)
        emit_qk(0, 1)
        for J in range(NJ):
            js = slice(J * JBLK, (J + 1) * JBLK)
            pidx = None
            ctx_tiles = []
            for p in range(2):
                pidx = J * 2 + p
                pv0 = ps_pv.tile([128, JBLK], F32, tag='pv')
                pv1 = ps_pv.tile([128, JBLK], F32, tag='pv')
                for kk in range(SK):
                    # QK two steps ahead, crossing pair boundaries so the
                    # next pair's logits are queued before this pair's tail
                    if kk + 2 < SK:
                        emit_qk(pidx, kk + 2)
                    else:
                        emit_qk(pidx + 1, kk + 2 - SK)
                    psl = psl_q.popleft()
                    weave_emit(2)
                    if (p == 0 and not weave and pend_state['g']
                            and kk in (11, 13, 15)):
                        pend_state['g'].pop(0)()
                    elif p == 1 and pend_state['g'] and kk % 2 == 0:
                        pend_state['g'].pop(0)()
                    ex = expool.tile([128, 2 * JBLK], BF16, tag='ex')
                    nc.scalar.activation(ex, psl, mybir.ActivationFunctionType.Exp)
                    if _DEBUG and J == 0 and p == 0 and kk == 0:
                        de = outpool.tile([128, 2 * JBLK], F32, tag='de')
                        nc.vector.tensor_copy(de, ex)
                        nc.sync.dma_start(out=dbg['d_ex'].ap(), in_=de)
                    for hh, pv in enumerate((pv0, pv1)):
                        hcol = 2 * p + hh
                        nc.tensor.matmul(
                            pv[0:DH + 1, :],
                            v_sb[kk][:, hcol, :],
                            ex[:, hh * JBLK:(hh + 1) * JBLK],
                            start=(kk == 0), stop=(kk == SK - 1))
                # normalize: ctxT[d, q] * (1/denom[q]) via partition broadcast
                ct = ctxpool.tile([128, JBLK], BF16, tag=f'ctx{p}')
                stage = []
                for hh, pv in enumerate((pv0, pv1)):
                    rawct = small.tile([128, JBLK], F32, tag='rawct')
                    nc.vector.tensor_copy(rawct[0:DH + 1, :], pv[0:DH + 1, :])
                    if _DEBUG and J == 0 and p == 0 and hh == 0:
                        dr = outpool.tile([128, JBLK], F32, tag='dr')
                        nc.vector.tensor_copy(dr[0:DH + 1, :], rawct[0:DH + 1, :])
                        nc.sync.dma_start(out=dbg['d_raw'].ap(), in_=dr)
                    rec = small.tile([128, JBLK], F32, tag='rec')
                    nc.vector.reciprocal_approx_fast(rec[0:DH + 1, :],
                                                     rawct[0:DH + 1, :])
                    bcs = small.tile([128, JBLK], F32, tag='bcs')
                    nc.sync.dma_start(out=bcs[0:1, :], in_=rec[DH:DH + 1, :])
                    bc = small.tile([128, JBLK], F32, tag='bc')
                    nc.gpsimd.partition_broadcast(bc[0:DH, :], bcs[0:1, :])
                    stage.append((rawct, bc))
                for hh, (rawct, bc) in enumerate(stage):
                    if hh == 0:
                        nc.vector.tensor_mul(ct[0:DH, :], rawct[0:DH, :], bc[0:DH, :])
                    else:
                        tmp = small.tile([128, JBLK], BF16, tag='tmp')
                        nc.vector.tensor_mul(tmp[0:DH, :], rawct[0:DH, :], bc[0:DH, :])
                        nc.sync.dma_start(out=ct[DH:128, :], in_=tmp[0:DH, :])
                if _DEBUG and J == 0 and p == 0:
                    dc = outpool.tile([128, JBLK], F32, tag='dc')
                    nc.vector.tensor_copy(dc, ct)
                    nc.sync.dma_start(out=dbg['d_ctx'].ap(), in_=dc)
                ctx_tiles.append(ct)
            pend_state['g'] = out_groups(J, ctx_tiles)
        weave_emit(len(weave))
        for grp in pend_state['g']:
            grp()

    nc.compile()
    return nc


def _get_nc():
    if 'nc' not in _CACHE:
        _CACHE['nc'] = _build()
    return _CACHE['nc']


def shard_inputs(x, y, bias, wq, wk, wv, wo):
    """Build the 8 per-core input maps from full inputs."""
    scale = (H // NH) ** -0.5
    wqs = (wq * scale).astype(np.float32)
    bf = ml_dtypes.bfloat16
    in_maps = []
    for c in range(NCORES):
        b = c // (NCORES // B)
        g = c % (NCORES // B)
        cols = slice(g * C, (g + 1) * C)
        eb = np.exp(bias[b, 0, 0, :].astype(np.float64)).astype(np.float32)
        in_maps.append({
            'xT': np.ascontiguousarray(x[b].T.astype(bf)),
            'yT': np.ascontiguousarray(y[b].T.astype(bf)),
            'wq': np.ascontiguousarray(wqs[:, cols].astype(bf)),
            'wk': np.ascontiguousarray(wk[:, cols].astype(bf)),
            'wv': np.ascontiguousarray(wv[:, cols].astype(bf)),
            'wo': np.ascontiguousarray(wo[cols, :].astype(bf)),
            'ebias': np.ascontiguousarray(eb.reshape(SK, 128).T),
        })
    return in_maps


def kernel(x, y, bias, wq, wk, wv, wo, _trace=False):
    x, y, bias = np.asarray(x), np.asarray(y), np.asarray(bias)
    wq, wk, wv, wo = (np.asarray(t) for t in (wq, wk, wv, wo))
    nc = _get_nc()
    in_maps = shard_inputs(x, y, bias, wq, wk, wv, wo)
    kw = {}
    if _trace:
        kw = dict(trace=True, stitch_traces=False)
    res = bass_utils.run_bass_kernel_spmd(nc, in_maps, core_ids=list(range(NCORES)), **kw)
    full = np.zeros((B, S, H), dtype=np.float64)
    for c in range(NCORES):
        full[c // (NCORES // B)] += res.results[c]['out'].astype(np.float64)
    if _trace:
        _CACHE['last_results'] = res
    return full.astype(np.float32)
